# revision 13
# baseline (speedup 1.0000x reference)
"""MoE (8 experts, top-2) Bass kernel for 8 trn2 NeuronCores.

Strategy: data-parallel over tokens. Each core gets T/8 = 2048 tokens and all
expert weights (bf16). On device, per core:
  phase 1: router logits (fp32 matmul) -> top-2 via max8 -> combine weights via
           sigmoid -> per-expert token counts/positions via triangular-matmul
           prefix sums -> scatter (token_id, weight) into per-expert position
           maps in DRAM (indirect DMA).
  phase 2: per expert: gather routed token rows (indirect DMA), transpose on
           PE, dense FFN gelu(x@Wfc+bfc)@Wproj+bproj in bf16 with fp32
           accumulate, scale rows by combine weight, write to DRAM.
  phase 3: per token: gather its two expert output rows, add, write output.

Host does only slicing/concat (and dtype staging of the weights to bf16).
"""

import math
import os
import sys

import numpy as np

for _p in ("/opt/trn_rl_repo", "/root/.axon_site/_ro/trn_rl_repo"):
    if os.path.isdir(_p) and _p not in sys.path:
        sys.path.insert(0, _p)

import ml_dtypes  # noqa: E402
import concourse.bass as bass  # noqa: E402
import concourse.mybir as mybir  # noqa: E402
import concourse.tile as tile  # noqa: E402
from concourse import bacc  # noqa: E402
from concourse.bass import IndirectOffsetOnAxis  # noqa: E402
from concourse.masks import make_identity, make_upper_triangular  # noqa: E402

F32 = mybir.dt.float32
BF16 = mybir.dt.bfloat16
I32 = mybir.dt.int32
AF = mybir.ActivationFunctionType
ALU = mybir.AluOpType
AX = mybir.AxisListType

N_CORES = 8
P = 128


def _chunks(total, step):
    out = []
    off = 0
    while off < total:
        w = min(step, total - off)
        out.append((off, w))
        off += w
    return out


def build_moe(TLOC, H, F, E, CAP, SLOP=128):
    """Build the per-core Bass program (SPMD: identical on all cores)."""
    assert TLOC % P == 0 and H % P == 0 and F % P == 0 and E == 8
    assert CAP % 64 == 0
    KH = H // P            # contraction chunks over H
    KF = F // P            # f-tiles (and stage-2 contraction chunks)
    NT = TLOC // P         # token tiles
    NPOS = E * CAP + SLOP  # rows in position-indexed buffers
    assert NPOS % P == 0
    SUBS = _chunks(CAP, P)        # c-subtiles within an expert's region
    MACROS = _chunks(CAP, 512)    # stage-1 moving-operand macro tiles
    HT = _chunks(H, 512)          # stage-2 output h-tiles

    nc = bacc.Bacc("TRN2", target_bir_lowering=False, debug=False,
                   enable_asserts=True, num_devices=N_CORES)

    xloc = nc.dram_tensor("xloc", [TLOC, H], F32, kind="ExternalInput")
    wr = nc.dram_tensor("wr", [H, E], F32, kind="ExternalInput")
    brr = nc.dram_tensor("brr", [1, E], F32, kind="ExternalInput")
    wfc = nc.dram_tensor("wfc", [E, H, F], BF16, kind="ExternalInput")
    bfc = nc.dram_tensor("bfc", [E, F], F32, kind="ExternalInput")
    wpj = nc.dram_tensor("wpj", [E, F, H], BF16, kind="ExternalInput")
    bpj = nc.dram_tensor("bpj", [E, H], BF16, kind="ExternalInput")
    out = nc.dram_tensor("out", [TLOC, H], F32, kind="ExternalOutput")

    tokmap = nc.dram_tensor("tokmap", [NPOS, 1], I32)
    wmap = nc.dram_tensor("wmap", [NPOS, 1], F32)
    ybuf = nc.dram_tensor("ybuf", [NPOS, H], F32)

    with tile.TileContext(nc) as tc:
        # ---------------- constants ----------------
        with tc.tile_pool(name="const", bufs=1) as cpool:
            id_f32 = cpool.tile([P, P], F32, tag="id_f32")
            make_identity(nc, id_f32)
            id_bf = cpool.tile([P, P], BF16, tag="id_bf")
            make_identity(nc, id_bf)
            u_incl = cpool.tile([P, P], F32, tag="u_incl")
            make_upper_triangular(nc, u_incl, val=1.0, diag=True)
            u_strict = cpool.tile([P, P], F32, tag="u_strict")
            make_upper_triangular(nc, u_strict, val=1.0, diag=False)
            ones_f = cpool.tile([1, P], F32, tag="ones_f")
            nc.gpsimd.memset(ones_f[:], 1.0)
            ones_col = cpool.tile([P, 1], F32, tag="ones_col")
            nc.gpsimd.memset(ones_col[:], 1.0)
            ones_bf = cpool.tile([1, P], BF16, tag="ones_bf")
            nc.gpsimd.memset(ones_bf[:], 1.0)
            zbias = cpool.tile([P, 1], F32, tag="zbias")
            nc.gpsimd.memset(zbias[:], 0.0)
            tid_base = cpool.tile([P, 1], I32, tag="tid_base")
            nc.gpsimd.iota(tid_base[:], pattern=[[0, 1]], base=0,
                           channel_multiplier=1)
            bases_i = cpool.tile([1, E], I32, tag="bases_i")
            nc.gpsimd.iota(bases_i[:], pattern=[[CAP, E]], base=-1,
                           channel_multiplier=0)
            bases = cpool.tile([1, E], F32, tag="bases")
            nc.vector.tensor_copy(out=bases[:], in_=bases_i[:])
            zrow_i = cpool.tile([P, NPOS // P], I32, tag="zrow_i")
            nc.gpsimd.memset(zrow_i[:], 0)
            zrow_f = cpool.tile([P, NPOS // P], F32, tag="zrow_f")
            nc.gpsimd.memset(zrow_f[:], 0.0)
            br_sb = cpool.tile([1, E], F32, tag="br_sb")
            nc.sync.dma_start(out=br_sb[:], in_=brr.ap()[:, :])

            # zero the position maps (pad slots -> token 0, weight 0)
            nc.sync.dma_start(
                out=tokmap.ap().rearrange("(a p) o -> p (a o)", p=P),
                in_=zrow_i[:])
            nc.sync.dma_start(
                out=wmap.ap().rearrange("(a p) o -> p (a o)", p=P),
                in_=zrow_f[:])

            # -------- persistent per-token-tile tiles (used in phase 3) ----
            with tc.tile_pool(name="keep", bufs=1) as keep:
                p1 = [keep.tile([P, 1], I32, tag=f"p1_{i}", name=f"p1_{i}") for i in range(NT)]
                p2 = [keep.tile([P, 1], I32, tag=f"p2_{i}", name=f"p2_{i}") for i in range(NT)]

                # ================= PHASE 1: router =================
                with tc.tile_pool(name="ph1", bufs=3) as ph1, \
                     tc.tile_pool(name="ph1k", bufs=1) as ph1k, \
                     tc.tile_pool(name="ps_tr", bufs=2, space="PSUM") as ps_tr, \
                     tc.tile_pool(name="ps_sm", bufs=1, space="PSUM") as ps_sm:
                    wr_sb = ph1k.tile([P, KH * E], F32, tag="wr_sb")
                    # wr [H, E] -> [128, KH*E] with (p, k*E+e) = Wr[k*128+p, e]
                    for k in range(KH):
                        nc.sync.dma_start(
                            out=wr_sb[:, k * E:(k + 1) * E],
                            in_=wr.ap()[k * P:(k + 1) * P, :])

                    eq1 = []
                    eq2 = []
                    msk = []
                    w1l = []
                    w2l = []
                    cnt_sb = ph1k.tile([NT, E], F32, tag="cnt_sb")
                    for i in range(NT):
                        x_t = ph1.tile([P, H], F32, tag="x_t")
                        nc.sync.dma_start(out=x_t[:],
                                          in_=xloc.ap()[i * P:(i + 1) * P, :])
                        lg_ps = ps_sm.tile([P, E], F32, tag="lg_ps")
                        for k in range(KH):
                            tp = ps_tr.tile([P, P], F32, tag="tp")
                            nc.tensor.transpose(
                                out=tp[:], in_=x_t[:, k * P:(k + 1) * P],
                                identity=id_f32[:])
                            xT = ph1.tile([P, P], F32, tag="xT")
                            nc.vector.tensor_copy(out=xT[:], in_=tp[:])
                            nc.tensor.matmul(
                                out=lg_ps[:], lhsT=xT[:],
                                rhs=wr_sb[:, k * E:(k + 1) * E],
                                start=(k == 0), stop=False)
                        nc.tensor.matmul(out=lg_ps[:], lhsT=ones_f[:, :P],
                                         rhs=br_sb[:], start=False, stop=True)
                        lg = ph1.tile([P, E], F32, tag="lg")
                        nc.vector.tensor_copy(out=lg[:], in_=lg_ps[:])
                        m8 = ph1.tile([P, 8], F32, tag="m8")
                        nc.vector.max(out=m8[:], in_=lg[:])
                        e1 = ph1k.tile([P, E], F32, tag=f"eq1_{i}", name=f"eq1_{i}")
                        nc.vector.tensor_tensor(
                            out=e1[:], in0=lg[:],
                            in1=m8[:, 0:1].to_broadcast([P, E]),
                            op=ALU.is_equal)
                        e2 = ph1k.tile([P, E], F32, tag=f"eq2_{i}", name=f"eq2_{i}")
                        nc.vector.tensor_tensor(
                            out=e2[:], in0=lg[:],
                            in1=m8[:, 1:2].to_broadcast([P, E]),
                            op=ALU.is_equal)
                        mk = ph1k.tile([P, E], F32, tag=f"msk_{i}", name=f"msk_{i}")
                        nc.vector.tensor_add(out=mk[:], in0=e1[:], in1=e2[:])
                        # combine weights: w1 = sigmoid(m1-m2), w2 = 1 - w1
                        dt_ = ph1.tile([P, 2], F32, tag="dt_")
                        nc.vector.tensor_sub(out=dt_[:, 0:1], in0=m8[:, 0:1],
                                             in1=m8[:, 1:2])
                        nc.vector.tensor_sub(out=dt_[:, 1:2], in0=m8[:, 1:2],
                                             in1=m8[:, 0:1])
                        wv = ph1k.tile([P, 2], F32, tag=f"wv_{i}", name=f"wv_{i}")
                        nc.scalar.activation(out=wv[:], in_=dt_[:], func=AF.Sigmoid,
                                             bias=zbias[:])
                        eq1.append(e1)
                        eq2.append(e2)
                        msk.append(mk)
                        w1l.append(wv)
                        w2l.append(wv)
                        # per-tile expert counts (matmul writes at partition 0;
                        # DMA moves the row to partition i of cnt_sb)
                        cnt_row = ps_sm.tile([1, E], F32, tag="cnt_row")
                        nc.tensor.matmul(out=cnt_row[:],
                                         lhsT=ones_col[:, 0:1],
                                         rhs=mk[:], start=True, stop=True)
                        cnt_row_sb = ph1.tile([1, E], F32, tag="cnt_row_sb")
                        nc.vector.tensor_copy(out=cnt_row_sb[:], in_=cnt_row[:])
                        nc.sync.dma_start(out=cnt_sb[i:i + 1, :],
                                          in_=cnt_row_sb[:])

                    off_ps = ps_sm.tile([NT, E], F32, tag="off_ps")
                    nc.tensor.matmul(out=off_ps[:], lhsT=u_strict[0:NT, 0:NT],
                                     rhs=cnt_sb[:], start=True, stop=False)
                    nc.tensor.matmul(out=off_ps[:], lhsT=ones_f[:, :NT],
                                     rhs=bases[:], start=False, stop=True)
                    off_sb = ph1.tile([NT, E], F32, tag="off_sb")
                    nc.vector.tensor_copy(out=off_sb[:], in_=off_ps[:])
                    off_i = []
                    for i in range(NT):
                        oi = ph1k.tile([1, E], F32, tag=f"off_{i}",
                                       name=f"off_{i}")
                        nc.sync.dma_start(out=oi[:], in_=off_sb[i:i + 1, :])
                        off_i.append(oi)

                    for i in range(NT):
                        pos_ps = ps_sm.tile([P, E], F32, tag="pos_ps")
                        nc.tensor.matmul(out=pos_ps[:], lhsT=u_incl[:],
                                         rhs=msk[i][:], start=True, stop=False)
                        nc.tensor.matmul(out=pos_ps[:], lhsT=ones_f[:, :P],
                                         rhs=off_i[i][0:1, :],
                                         start=False, stop=True)
                        pos = ph1.tile([P, E], F32, tag="pos")
                        nc.vector.tensor_copy(out=pos[:], in_=pos_ps[:])
                        tmp = ph1.tile([P, E], F32, tag="tmp")
                        d1f = ph1.tile([P, 1], F32, tag="d1f")
                        d2f = ph1.tile([P, 1], F32, tag="d2f")
                        nc.vector.tensor_mul(out=tmp[:], in0=eq1[i][:], in1=pos[:])
                        nc.vector.reduce_sum(out=d1f[:], in_=tmp[:], axis=AX.X)
                        nc.vector.tensor_mul(out=tmp[:], in0=eq2[i][:], in1=pos[:])
                        nc.vector.reduce_sum(out=d2f[:], in_=tmp[:], axis=AX.X)
                        nc.vector.tensor_copy(out=p1[i][:], in_=d1f[:])
                        nc.vector.tensor_copy(out=p2[i][:], in_=d2f[:])
                        tid = ph1.tile([P, 1], I32, tag="tid")
                        nc.vector.tensor_scalar_add(tid[:], tid_base[:], i * P)
                        # scatter token ids and combine weights to positions
                        nc.gpsimd.indirect_dma_start(
                            out=tokmap.ap(), out_offset=IndirectOffsetOnAxis(
                                ap=p1[i][:, 0:1], axis=0),
                            in_=tid[:, 0:1], in_offset=None)
                        nc.gpsimd.indirect_dma_start(
                            out=tokmap.ap(), out_offset=IndirectOffsetOnAxis(
                                ap=p2[i][:, 0:1], axis=0),
                            in_=tid[:, 0:1], in_offset=None)
                        nc.gpsimd.indirect_dma_start(
                            out=wmap.ap(), out_offset=IndirectOffsetOnAxis(
                                ap=p1[i][:, 0:1], axis=0),
                            in_=w1l[i][:, 0:1], in_offset=None)
                        nc.gpsimd.indirect_dma_start(
                            out=wmap.ap(), out_offset=IndirectOffsetOnAxis(
                                ap=p2[i][:, 0:1], axis=0),
                            in_=w2l[i][:, 1:2], in_offset=None)

                # ================= PHASE 2: expert FFN =================
                with tc.tile_pool(name="wcache", bufs=2) as wc, \
                     tc.tile_pool(name="ph2", bufs=3) as ph2, \
                     tc.tile_pool(name="xet", bufs=2) as xetp, \
                     tc.tile_pool(name="h1t", bufs=1) as h1tp, \
                     tc.tile_pool(name="ysb", bufs=1) as ysbp, \
                     tc.tile_pool(name="wtk", bufs=2) as wtkp, \
                     tc.tile_pool(name="ps_tr2", bufs=1, space="PSUM") as ps_tr2, \
                     tc.tile_pool(name="ps_s1", bufs=2, space="PSUM") as ps_s1, \
                     tc.tile_pool(name="ps_y", bufs=1, space="PSUM") as ps_y:
                    for e in range(E):
                        wfc_k = []
                        for k in range(KH):
                            wk = wc.tile([P, F], BF16, tag=f"wfc{k}", name=f"wfc{k}")
                            nc.sync.dma_start(
                                out=wk[:], in_=wfc.ap()[e, k * P:(k + 1) * P, :])
                            wfc_k.append(wk)
                        bfc_sb = wc.tile([P, KF], F32, tag="bfc_sb")
                        nc.sync.dma_start(
                            out=bfc_sb[:],
                            in_=bfc.ap()[e:e + 1, :].rearrange(
                                "o (a p) -> p (o a)", p=P))
                        bpj_sb = wc.tile([1, H], BF16, tag="bpj_sb")
                        nc.sync.dma_start(out=bpj_sb[:], in_=bpj.ap()[e:e + 1, :])

                        xet = [xetp.tile([P, CAP], BF16, tag=f"xet{k}", name=f"xet{k}")
                               for k in range(KH)]
                        w_t = []
                        for ci, (cs, cw) in enumerate(SUBS):
                            tok = ph2.tile([P, 1], I32, tag="tok")
                            nc.sync.dma_start(
                                out=tok[0:cw, :],
                                in_=tokmap.ap()[e * CAP + cs:e * CAP + cs + cw, :])
                            wt = wtkp.tile([P, 1], F32, tag=f"wt{ci}", name=f"wt{ci}")
                            nc.sync.dma_start(
                                out=wt[0:cw, :],
                                in_=wmap.ap()[e * CAP + cs:e * CAP + cs + cw, :])
                            w_t.append(wt)
                            xg = ph2.tile([P, H], F32, tag="xg")
                            nc.gpsimd.indirect_dma_start(
                                out=xg[0:cw, :], out_offset=None,
                                in_=xloc.ap(),
                                in_offset=IndirectOffsetOnAxis(
                                    ap=tok[0:cw, 0:1], axis=0))
                            xgb = ph2.tile([P, H], BF16, tag="xgb")
                            nc.vector.tensor_copy(out=xgb[0:cw, :], in_=xg[0:cw, :])
                            for k in range(KH):
                                tp2 = ps_tr2.tile([P, P], BF16, tag="tp2")
                                nc.tensor.transpose(
                                    out=tp2[:, 0:cw],
                                    in_=xgb[0:cw, k * P:(k + 1) * P],
                                    identity=id_bf[0:cw, 0:cw])
                                nc.vector.tensor_copy(out=xet[k][:, cs:cs + cw],
                                                      in_=tp2[:, 0:cw])

                        h1t = [h1tp.tile([P, CAP], BF16, tag=f"h1t{ft}", name=f"h1t{ft}")
                               for ft in range(KF)]
                        for (ms, mw) in MACROS:
                            for ft in range(KF):
                                ps1 = ps_s1.tile([P, 512], F32, tag="ps1")
                                for k in range(KH):
                                    nc.tensor.matmul(
                                        out=ps1[:, 0:mw],
                                        lhsT=wfc_k[k][:, ft * P:(ft + 1) * P],
                                        rhs=xet[k][:, ms:ms + mw],
                                        start=(k == 0), stop=(k == KH - 1))
                                nc.scalar.activation(
                                    out=h1t[ft][:, ms:ms + mw], in_=ps1[:, 0:mw],
                                    func=AF.Gelu_apprx_tanh,
                                    bias=bfc_sb[:, ft:ft + 1])

                        ysb = [ysbp.tile([P, H], F32, tag=f"ysb{ci}", name=f"ysb{ci}")
                               for ci in range(len(SUBS))]
                        for (hs, hw) in HT:
                            psy = [ps_y.tile([P, hw], F32, tag=f"psy{ci}", name=f"psy{ci}")
                                   for ci in range(len(SUBS))]
                            for k in range(KF):
                                wp = ph2.tile([P, 512], BF16, tag="wp")
                                nc.sync.dma_start(
                                    out=wp[:, 0:hw],
                                    in_=wpj.ap()[e, k * P:(k + 1) * P, hs:hs + hw])
                                for ci, (cs, cw) in enumerate(SUBS):
                                    nc.tensor.matmul(
                                        out=psy[ci][0:cw, 0:hw],
                                        lhsT=h1t[k][:, cs:cs + cw],
                                        rhs=wp[:, 0:hw],
                                        start=(k == 0), stop=False)
                            for ci, (cs, cw) in enumerate(SUBS):
                                nc.tensor.matmul(
                                    out=psy[ci][0:cw, 0:hw],
                                    lhsT=ones_bf[:, 0:cw],
                                    rhs=bpj_sb[:, hs:hs + hw],
                                    start=False, stop=True)
                                nc.vector.tensor_scalar_mul(
                                    ysb[ci][0:cw, hs:hs + hw],
                                    psy[ci][0:cw, 0:hw],
                                    w_t[ci][0:cw, 0:1])
                        for ci, (cs, cw) in enumerate(SUBS):
                            nc.sync.dma_start(
                                out=ybuf.ap()[e * CAP + cs:e * CAP + cs + cw, :],
                                in_=ysb[ci][0:cw, :])

                # ================= PHASE 3: combine =================
                with tc.tile_pool(name="ph3", bufs=4) as ph3:
                    for i in range(NT):
                        y1 = ph3.tile([P, H], F32, tag="y1")
                        y2 = ph3.tile([P, H], F32, tag="y2")
                        nc.gpsimd.indirect_dma_start(
                            out=y1[:], out_offset=None, in_=ybuf.ap(),
                            in_offset=IndirectOffsetOnAxis(ap=p1[i][:, 0:1],
                                                           axis=0))
                        nc.gpsimd.indirect_dma_start(
                            out=y2[:], out_offset=None, in_=ybuf.ap(),
                            in_offset=IndirectOffsetOnAxis(ap=p2[i][:, 0:1],
                                                           axis=0))
                        nc.vector.tensor_add(out=y1[:], in0=y1[:], in1=y2[:])
                        nc.sync.dma_start(out=out.ap()[i * P:(i + 1) * P, :],
                                          in_=y1[:])

    nc.compile()
    return nc


# ---------------------------------------------------------------------------
_BUILD_CACHE = {}
_LAST_IN_MAPS = None


def _get_built(TLOC, H, F, E, CAP):
    key = (TLOC, H, F, E, CAP)
    if key not in _BUILD_CACHE:
        _BUILD_CACHE[key] = build_moe(TLOC, H, F, E, CAP)
    return _BUILD_CACHE[key]


def kernel(hidden_states, Wr, br, Wfc, bfc, Wproj, bproj):
    from concourse.bass_utils import run_bass_kernel_spmd

    hs = np.ascontiguousarray(np.asarray(hidden_states, dtype=np.float32))
    Wr = np.ascontiguousarray(np.asarray(Wr, dtype=np.float32))
    br = np.ascontiguousarray(np.asarray(br, dtype=np.float32))
    Wfc = np.asarray(Wfc, dtype=np.float32)
    bfc = np.ascontiguousarray(np.asarray(bfc, dtype=np.float32))
    Wproj = np.asarray(Wproj, dtype=np.float32)
    bproj = np.asarray(bproj, dtype=np.float32)

    B, S, H = hs.shape
    E, H2, F = Wfc.shape
    assert H2 == H
    T = B * S
    assert T % N_CORES == 0
    TLOC = T // N_CORES
    x = hs.reshape(T, H)

    # host-side routing peek ONLY to pick the compile-time capacity
    logits = x @ Wr + br[None, :]
    top2 = np.argpartition(-logits, 2, axis=1)[:, :2]
    maxcnt = 0
    for c in range(N_CORES):
        sl = top2[c * TLOC:(c + 1) * TLOC]
        cnts = np.bincount(sl.ravel(), minlength=E)
        maxcnt = max(maxcnt, int(cnts.max()))
    CAP = max(128, int(math.ceil((maxcnt + 8) / 64.0)) * 64)

    nc = _get_built(TLOC, H, F, E, CAP)

    wfc_bf = np.ascontiguousarray(Wfc.astype(ml_dtypes.bfloat16))
    wpj_bf = np.ascontiguousarray(Wproj.astype(ml_dtypes.bfloat16))
    bpj_bf = np.ascontiguousarray(bproj.astype(ml_dtypes.bfloat16))
    br_row = np.ascontiguousarray(br.reshape(1, E))

    in_maps = []
    for c in range(N_CORES):
        in_maps.append({
            "xloc": np.ascontiguousarray(x[c * TLOC:(c + 1) * TLOC]),
            "wr": Wr,
            "brr": br_row,
            "wfc": wfc_bf,
            "bfc": bfc,
            "wpj": wpj_bf,
            "bpj": bpj_bf,
        })

    global _LAST_IN_MAPS
    _LAST_IN_MAPS = in_maps

    res = run_bass_kernel_spmd(nc, in_maps, core_ids=list(range(N_CORES)))
    outs = [res.results[c]["out"] for c in range(N_CORES)]
    return np.concatenate(outs, axis=0).reshape(B, S, H).astype(np.float32)
